# revision 1
# baseline (speedup 1.0000x reference)
"""Trainium2 Bass kernel for nn_CausalSelfAttention_40810779247124.

Head-sharded (tensor-parallel) causal self-attention prefill across 8
NeuronCores: 2 heads per core. Per core:

  phase 1: QKV projection for its 2 heads, outputs in [e, tok] layout
           (contraction-friendly), Q/K kept resident in SBUF, V
           PE-transposed to [tok, e] and kept resident in SBUF.
  phase 2: attention computed transposed: scoresT[t,s] = K.T @ Q (both
           operands already have Dh on partitions), exp on ScalarE,
           denominator via ones-matmul (partition-sum on PE),
           wvT[Dh,s] = V.T @ P.T accumulated on PE.  wvT staged to DRAM.
  phase 3: output projection partial: out[tok,:] += wvT.T @ w_outT for
           this core's d-slice.  The all-reduce over cores is done on
           the host during unsharding (sum of 8 partials).

Causality is exploited (t>s blocks skipped); the host verifies that
mask/cache_pos match the causal-prefill pattern and falls back to a
numpy reference otherwise.  All matmul operands use float32r (full-rate
fp32 matmul mode on TRN2).
"""

import sys

sys.path.insert(0, "/opt/trn_rl_repo")

import numpy as np

B = 2
S = 2048
T = 4096
NS = 2048          # n_state
H = 16
DH = 128
NCORES = 8
HPC = H // NCORES  # heads per core = 2
DPC = HPC * DH     # d-slice per core = 256
TOK = B * S        # 4096 tokens across batches
SCALE = 1.0 / float(np.sqrt(DH))

_CACHED = {}


def _build_program():
    import concourse.bacc as bacc
    import concourse.bass as bass
    import concourse.tile as tile
    from concourse import mybir
    f32r = mybir.dt.float32r
    f32 = mybir.dt.float32

    nc = bacc.Bacc()

    xT = nc.dram_tensor("xT", [NS, TOK], f32r, kind="ExternalInput")
    wT = nc.dram_tensor("wT", [NS, 6 * DH], f32r, kind="ExternalInput")
    woutT = nc.dram_tensor("woutT", [DPC, NS], f32r, kind="ExternalInput")
    cmask = nc.dram_tensor("cmask", [DH, 4 * 512 + 256], f32r, kind="ExternalInput")
    outp = nc.dram_tensor("outp", [TOK, NS], f32, kind="ExternalOutput")

    NT = TOK // 512   # 8 tok-tiles of 512
    NK = NS // 128    # 16 contraction chunks

    with tile.TileContext(nc) as tc:
        with (
            tc.tile_pool(name="constp", bufs=1) as constp,
            tc.tile_pool(name="vresp", bufs=1) as vresp,
            tc.tile_pool(name="dramp", bufs=1, space="DRAM") as dramp,
        ):
            cmask_sb = constp.tile([DH, 4 * 512 + 256], f32r)
            ones_col = cmask_sb[:, 2176:2177]
            ones_row = cmask_sb[0:1, 2176:2304]

            # V resident across phases 1-2: v_res[p, c, e] = V[c*128+p, e]
            v_res = vresp.tile([128, TOK // 128, DPC], f32r)

            # wvT staging through DRAM between phases 2 and 3
            wvn_d = dramp.tile([B * DPC, S], f32r)

            with tc.tile_pool(name="qkresp", bufs=1) as qkresp:
                # Q,K resident [e-block(q0,q1,k0,k1), tok]
                qk_res = qkresp.tile([128, 4, TOK], f32r)

                # ---------------- phase 1: QKV projection ----------------
                with (
                    tc.tile_pool(name="wp", bufs=1) as wp,
                    tc.tile_pool(name="xp", bufs=3) as xp,
                    tc.tile_pool(name="qkv_ps", bufs=4, space="PSUM") as qkv_ps,
                    tc.tile_pool(name="v_ps", bufs=4, space="PSUM") as v_ps,
                ):
                    w_sb = wp.tile([128, NK, 6 * DH], f32r)

                    for a in range(NT):
                        pss = [
                            qkv_ps.tile([128, 512], f32, tag="qkv", name=f"qkv{m}")
                            for m in range(4)
                        ]
                        vps = [
                            v_ps.tile([128, 256], f32, tag="vps", name=f"vps{t}")
                            for t in range(4)
                        ]
                        for half in range(2):
                            x_sb = xp.tile([128, NK // 2, 512], f32r, tag="x_sb")
                            for kc in range(NK // 2):
                                nc.scalar.dma_start(
                                    out=x_sb[:, kc, :],
                                    in_=xT[
                                        1024 * half + 128 * kc : 1024 * half
                                        + 128 * (kc + 1),
                                        512 * a : 512 * (a + 1),
                                    ],
                                )
                            for kc in range(NK // 2):
                                kk = half * (NK // 2) + kc
                                if a == 0:
                                    if kk == 0:
                                        for mm in range(6):
                                            nc.sync.dma_start(
                                                out=w_sb[
                                                    :, kk, 128 * mm : 128 * (mm + 1)
                                                ],
                                                in_=wT[
                                                    128 * kk : 128 * (kk + 1),
                                                    128 * mm : 128 * (mm + 1),
                                                ],
                                            )
                                    else:
                                        nc.sync.dma_start(
                                            out=w_sb[:, kk, :],
                                            in_=wT[128 * kk : 128 * (kk + 1), :],
                                        )
                                for m in range(4):
                                    nc.tensor.matmul(
                                        pss[m],
                                        w_sb[:, kk, 128 * m : 128 * (m + 1)],
                                        x_sb[:, kc, :],
                                        start=(kk == 0),
                                        stop=(kk == NK - 1),
                                    )
                                for t in range(4):
                                    nc.tensor.matmul(
                                        vps[t],
                                        x_sb[:, kc, 128 * t : 128 * (t + 1)],
                                        w_sb[:, kk, 512:768],
                                        start=(kk == 0),
                                        stop=(kk == NK - 1),
                                    )
                        for m in range(4):
                            # Q/K to resident SBUF in [e, tok] layout
                            nc.vector.tensor_copy(
                                out=qk_res[:, m, 512 * a : 512 * (a + 1)],
                                in_=pss[m],
                            )
                        for t in range(4):
                            nc.vector.tensor_copy(
                                out=v_res[:, 4 * a + t, :], in_=vps[t]
                            )

                    nc.scalar.dma_start(
                        out=cmask_sb[:, 0:2048], in_=cmask[:, 0:2048]
                    )
                    nc.scalar.dma_start(
                        out=cmask_sb[:, 2176:2304], in_=cmask[:, 2176:2304]
                    )

                # ------- phases 2+3: attention + out-projection per batch -------
                with (
                    tc.tile_pool(name="woutp", bufs=1) as woutp,
                    tc.tile_pool(name="ptp", bufs=4) as ptp,
                    tc.tile_pool(name="zrp", bufs=2) as zrp,
                    tc.tile_pool(name="wvnp", bufs=5) as wvnp,
                    tc.tile_pool(name="ostage", bufs=3) as ostage,
                    tc.tile_pool(name="sc_ps", bufs=2, space="PSUM") as sc_ps,
                    tc.tile_pool(name="wv_ps", bufs=2, space="PSUM") as wv_ps,
                    tc.tile_pool(name="z_ps", bufs=2, space="PSUM") as z_ps,
                    tc.tile_pool(name="o_ps", bufs=2, space="PSUM") as o_ps,
                ):
                    wout_sb = woutp.tile([128, HPC, NS], f32r)
                    for h in range(HPC):
                        nc.sync.dma_start(
                            out=wout_sb[:, h, :],
                            in_=woutT[128 * h : 128 * (h + 1), :],
                        )
                    def finalize(fin):
                        wv, z, wvn, ast = fin
                        zr = zrp.tile([1, 512], f32r, tag="zr")
                        with nc.allow_low_precision(
                            reason="f32r is bit-identical to f32"
                        ):
                            nc.vector.reciprocal(out=zr, in_=z)
                        zb = z_ps.tile([128, 512], f32, tag="z")
                        nc.tensor.matmul(zb, ones_row, zr, start=True, stop=True)
                        zbs = zrp.tile([128, 512], f32r, tag="zbs")
                        nc.vector.tensor_copy(out=zbs, in_=zb)
                        nc.vector.tensor_mul(
                            wvn[:, 512 * ast : 512 * (ast + 1)], wv, zbs
                        )

                    for b in range(B):
                        wvn_tiles = []
                        for h in range(HPC):
                            q_sb = qk_res[:, h, S * b : S * (b + 1)]
                            k_sb = qk_res[:, 2 + h, S * b : S * (b + 1)]
                            wvn = wvnp.tile([128, S], f32r, tag="wvn")
                            wvn_tiles.append(wvn)
                            for ast in range(S // 512):
                                nj = 4 * ast + 4  # causal t-blocks
                                wv = wv_ps.tile([128, 512], f32, tag="wv")
                                z_full = z_ps.tile([128, 512], f32, tag="z")
                                z = z_full[0:1, :]
                                for j in range(nj):
                                    sc = sc_ps.tile([128, 512], f32, tag="sc")
                                    nc.tensor.matmul(
                                        sc,
                                        k_sb[:, 128 * j : 128 * (j + 1)],
                                        q_sb[:, 512 * ast : 512 * (ast + 1)],
                                        start=True,
                                        stop=True,
                                    )
                                    pt = ptp.tile([128, 512], f32r, tag="pt")
                                    nc.scalar.activation(
                                        out=pt,
                                        in_=sc,
                                        func=mybir.ActivationFunctionType.Exp,
                                        scale=SCALE,
                                    )
                                    p = j - 4 * ast
                                    if p >= 0:
                                        nc.vector.tensor_mul(
                                            pt,
                                            pt,
                                            cmask_sb[:, 512 * p : 512 * (p + 1)],
                                        )
                                    nc.tensor.matmul(
                                        z,
                                        ones_col,
                                        pt,
                                        start=(j == 0),
                                        stop=(j == nj - 1),
                                    )
                                    nc.tensor.matmul(
                                        wv,
                                        v_res[
                                            :, 16 * b + j, 128 * h : 128 * (h + 1)
                                        ],
                                        pt,
                                        start=(j == 0),
                                        stop=(j == nj - 1),
                                    )
                                finalize((wv, z, wvn, ast))
                        # out-projection for this batch (wvn of both heads)
                        for tk in range(S // 128):
                            ost = ostage.tile([128, NS], f32, tag="ost")
                            for n in range(NS // 512):
                                ops = o_ps.tile([128, 512], f32, tag="ops")
                                for h in range(HPC):
                                    nc.tensor.matmul(
                                        ops,
                                        wvn_tiles[h][:, 128 * tk : 128 * (tk + 1)],
                                        wout_sb[:, h, 512 * n : 512 * (n + 1)],
                                        start=(h == 0),
                                        stop=(h == HPC - 1),
                                    )
                                nc.vector.tensor_copy(
                                    out=ost[:, 512 * n : 512 * (n + 1)], in_=ops
                                )
                            for hh in range(2):
                                nc.sync.dma_start(
                                    out=outp[
                                        S * b + 128 * tk : S * b + 128 * (tk + 1),
                                        1024 * hh : 1024 * (hh + 1),
                                    ],
                                    in_=ost[:, 1024 * hh : 1024 * (hh + 1)],
                                )

    nc.compile()
    return nc


def _causal_fastpath_ok(mask, cache_pos):
    if cache_pos.shape != (S,) or not np.array_equal(
        np.asarray(cache_pos), np.arange(S, dtype=np.int64).astype(cache_pos.dtype)
    ):
        return False
    m = np.asarray(mask).reshape(S, T)
    rows = np.arange(S)[:, None]
    cols = np.arange(T)[None, :]
    return np.array_equal(m, cols <= rows)


def _numpy_fallback(input_ids, mask, cache_pos, w_qkv, w_out, k_cache, v_cache):
    x = np.asarray(input_ids, dtype=np.float32)
    qkv = np.einsum("bsd,ed->bse", x, np.asarray(w_qkv, np.float32))
    q, k, v = np.split(qkv, 3, axis=-1)

    def heads(t):
        return t.reshape(B, S, H, DH).transpose(0, 2, 1, 3)

    q, k, v = heads(q), heads(k), heads(v)
    kf = np.array(k_cache, np.float32)
    vf = np.array(v_cache, np.float32)
    kf[:, :, np.asarray(cache_pos)] = k
    vf[:, :, np.asarray(cache_pos)] = v
    sc = np.einsum("bhsd,bhtd->bhst", q, kf) * SCALE
    sc = np.where(np.asarray(mask), sc, np.finfo(np.float32).min)
    sc = sc - sc.max(axis=-1, keepdims=True)
    p = np.exp(sc)
    p = p / p.sum(axis=-1, keepdims=True)
    wv = np.einsum("bhst,bhtd->bhsd", p, vf)
    wv = wv.transpose(0, 2, 1, 3).reshape(B, S, NS)
    return np.einsum("bsd,ed->bse", wv, np.asarray(w_out, np.float32))


def _build_cmask_host():
    # 4 multiplicative mask tiles [128, 512] laid side by side: tile p is
    # applied to scoresT block (t rows) against an s-tile of width 512 when
    # the t-block is the p-th 128-strip inside that s-tile.
    t = np.arange(128)[:, None]
    s = np.arange(512)[None, :]
    tiles = []
    for p in range(4):
        tiles.append(((s - 128 * p) >= t).astype(np.float32))
    # trailing constant blocks: [identity(128) | ones(128)]
    tiles.append(np.eye(128, dtype=np.float32))
    tiles.append(np.ones((128, 128), dtype=np.float32))
    return np.concatenate(tiles, axis=1)  # [128, 2304]


def _run_on_device(in_maps, trace=False):
    from concourse.bass_utils import run_bass_kernel_spmd

    if "nc" not in _CACHED:
        _CACHED["nc"] = _build_program()
    nc = _CACHED["nc"]
    return run_bass_kernel_spmd(
        nc, in_maps, core_ids=list(range(NCORES)), trace=trace
    )


def _prep_in_maps(input_ids, w_qkv, w_out):
    x2d = np.ascontiguousarray(
        np.asarray(input_ids, np.float32).reshape(TOK, NS).T
    )  # [NS, TOK]
    cm = _build_cmask_host()
    wq = np.asarray(w_qkv, np.float32)
    wo = np.asarray(w_out, np.float32)
    in_maps = []
    for c in range(NCORES):
        lo, hi = c * DPC, (c + 1) * DPC
        w_slice = np.concatenate(
            [wq[lo:hi], wq[NS + lo : NS + hi], wq[2 * NS + lo : 2 * NS + hi]],
            axis=0,
        )  # [768, NS] (q,k,v rows for this core's heads)
        wT_c = np.ascontiguousarray(w_slice.T)        # [NS, 768]
        woutT_c = np.ascontiguousarray(wo[:, lo:hi].T)  # [DPC, NS]
        in_maps.append({"xT": x2d, "wT": wT_c, "woutT": woutT_c, "cmask": cm})
    return in_maps


def kernel(input_ids, mask, cache_pos, w_qkv, w_out, k_cache, v_cache):
    if not _causal_fastpath_ok(mask, cache_pos):
        return _numpy_fallback(
            input_ids, mask, cache_pos, w_qkv, w_out, k_cache, v_cache
        )
    in_maps = _prep_in_maps(input_ids, w_qkv, w_out)
    res = _run_on_device(in_maps)
    out = np.zeros((TOK, NS), np.float32)
    for r in res.results:
        out += r["outp"]
    return out.reshape(B, S, NS)



# revision 20
# speedup vs baseline: 1.2189x; 1.2189x over previous
"""Trainium2 Bass kernel for nn_CausalSelfAttention_40810779247124.

Head-sharded (tensor-parallel) causal self-attention prefill across 8
NeuronCores: 2 heads per core, both batches on every core.  All matmul
operands are fp16 (fp32 PSUM accumulation), which runs at the full
1 row/cycle PE rate while halving DMA traffic and enabling the DVE
2x mode for elementwise work.

Per core:
  phase 1: QKV projection.  Q,K kept resident in SBUF in [e, tok]
           layout; V resident in [tok, e] layout (both fp16).
  phase 2: attention, transposed: scoresT[t,s] = K.T @ Q on PE,
           exp on ScalarE (PSUM f32 -> SBUF fp16), softmax denominator
           accumulated on DVE (fp16 2x adds) + one ones-matmul
           partition-reduce/broadcast per s-tile, wvT[e,s] = V.T @ P.T
           accumulated in PSUM.  Causal structure exploited at 128-token
           granularity via sliced matmuls on the diagonal s-tiles.
  phase 3: output projection partial for this core's 256-dim slice,
           interleaved with attention per 512-token group; the
           all-reduce over cores happens on the host (sum of 8 fp16
           partials in f32) during unsharding.

The host verifies that mask/cache_pos match the causal-prefill pattern
and falls back to a numpy reference otherwise.
"""

import sys

sys.path.insert(0, "/opt/trn_rl_repo")

import numpy as np

B = 2
S = 2048
T = 4096
NS = 2048          # n_state
H = 16
DH = 128
NCORES = 8
HPC = H // NCORES  # heads per core = 2
DPC = HPC * DH     # d-slice per core = 256
TOK = B * S        # 4096 tokens across batches
SCALE = 1.0 / float(np.sqrt(DH))

_CACHED = {}


def _build_program(debug=False):
    import concourse.bacc as bacc
    import concourse.bass as bass
    import concourse.tile as tile
    from concourse import mybir
    f16 = mybir.dt.float16
    f32 = mybir.dt.float32

    nc = bacc.Bacc()

    xT = nc.dram_tensor("xT", [NS, TOK], f16, kind="ExternalInput")
    wT = nc.dram_tensor("wT", [NS, 6 * DH], f16, kind="ExternalInput")
    woutT = nc.dram_tensor("woutT", [DPC, NS], f16, kind="ExternalInput")
    cmask = nc.dram_tensor("cmask", [DH, 256], f16, kind="ExternalInput")
    outp = nc.dram_tensor("outp", [TOK, NS], f16, kind="ExternalOutput")
    if debug:
        dbg_qk = nc.dram_tensor("dbg_qk", [128, 4 * TOK], f16, kind="ExternalOutput")
        dbg_v = nc.dram_tensor("dbg_v", [128, (TOK // 128) * DPC], f16, kind="ExternalOutput")
        dbg_wvn = nc.dram_tensor("dbg_wvn", [128, B * HPC * S], f16, kind="ExternalOutput")

    NT = TOK // 512   # 8 tok-tiles of 512
    NK = NS // 128    # 16 contraction chunks

    with tile.TileContext(nc) as tc:
        with (
            tc.tile_pool(name="constp", bufs=1) as constp,
            tc.tile_pool(name="vresp", bufs=1) as vresp,
            tc.tile_pool(name="qkresp", bufs=1) as qkresp,
            tc.tile_pool(name="wp", bufs=1) as wp,
            tc.tile_pool(name="woutp", bufs=1) as woutp,
        ):
            cm_sb = constp.tile([DH, 256], f16)
            tri = cm_sb[:, 0:128]     # tri[t, s'] = s' >= t
            ones_mat = cm_sb[:, 128:256]

            # residents: V[tok, e] (32 chunks of 128 tok), Q/K [e, tok]
            v_res = vresp.tile([128, TOK // 128, DPC], f16)
            qk_res = qkresp.tile([128, 4, TOK], f16)  # m = q0,q1,k0,k1
            w_sb = wp.tile([128, NK, 6 * DH], f16)
            wout_sb = woutp.tile([128, HPC, NS], f16)

            # ---------------- phase 1: QKV projection ----------------
            with (
                tc.tile_pool(name="xp", bufs=2) as xp,
                tc.tile_pool(name="qkv_ps", bufs=4, space="PSUM") as qkv_ps,
                tc.tile_pool(name="v_ps", bufs=4, space="PSUM") as v_ps,
            ):
                for a in range(NT):
                    x_sb = xp.tile([128, NK, 512], f16, tag="x_sb")
                    # x DMAs are [128, 4, 512]; the host pre-permutes xT rows
                    # (see _prep_in_maps) so group g row 4*p+k holds logical
                    # row 128*k+p and the tile lands as x_sb[p, kc, c] =
                    # x[128*kc+p, col].
                    if a == 0:
                        # interleave x/w chunk DMAs so the kc=0 operands land
                        # first and the kc-outer compute below starts early
                        for q in range(4):
                            nc.sync.dma_start(
                                out=x_sb[:, 4 * q : 4 * (q + 1), :],
                                in_=xT[512 * q : 512 * (q + 1), 0:512],
                            )
                            for kc in range(4 * q, 4 * (q + 1)):
                                nc.sync.dma_start(
                                    out=w_sb[:, kc, :],
                                    in_=wT[128 * kc : 128 * (kc + 1), :],
                                )
                        nc.sync.dma_start(out=cm_sb, in_=cmask[:, :])
                        for h in range(HPC):
                            nc.sync.dma_start(
                                out=wout_sb[:, h, :],
                                in_=woutT[128 * h : 128 * (h + 1), :],
                            )
                        # kc-outer so each chunk's matmuls only need that
                        # chunk's x/w DMA
                        pss4 = [
                            qkv_ps.tile([128, 512], f32, tag="qkv", name=f"pss{m}")
                            for m in range(4)
                        ]
                        vps4 = [
                            v_ps.tile([128, 256], f32, tag="vps", name=f"vps{t}")
                            for t in range(4)
                        ]
                        for kc in range(NK):
                            for m in range(4):
                                nc.tensor.matmul(
                                    pss4[m],
                                    w_sb[:, kc, 128 * m : 128 * (m + 1)],
                                    x_sb[:, kc, :],
                                    start=(kc == 0),
                                    stop=(kc == NK - 1),
                                )
                            for t in range(4):
                                nc.tensor.matmul(
                                    vps4[t],
                                    x_sb[:, kc, 128 * t : 128 * (t + 1)],
                                    w_sb[:, kc, 512:768],
                                    start=(kc == 0),
                                    stop=(kc == NK - 1),
                                )
                        for m in range(4):
                            nc.scalar.activation(
                                out=qk_res[:, m, 0:512],
                                in_=pss4[m],
                                func=mybir.ActivationFunctionType.Copy,
                            )
                        for t in range(4):
                            nc.scalar.activation(
                                out=v_res[:, t, :],
                                in_=vps4[t],
                                func=mybir.ActivationFunctionType.Copy,
                            )
                        continue
                    for q in range(4):
                        nc.sync.dma_start(
                            out=x_sb[:, 4 * q : 4 * (q + 1), :],
                            in_=xT[
                                512 * q : 512 * (q + 1),
                                512 * a : 512 * (a + 1),
                            ],
                        )
                    # Q,K: out [e(128), tok(512)] per m-tile
                    for m in range(4):
                        pss = qkv_ps.tile([128, 512], f32, tag="qkv")
                        for kc in range(NK):
                            nc.tensor.matmul(
                                pss,
                                w_sb[:, kc, 128 * m : 128 * (m + 1)],
                                x_sb[:, kc, :],
                                start=(kc == 0),
                                stop=(kc == NK - 1),
                            )
                        nc.scalar.activation(
                            out=qk_res[:, m, 512 * a : 512 * (a + 1)],
                            in_=pss,
                            func=mybir.ActivationFunctionType.Copy,
                        )
                    # V: out [tok(128), e(256)], one bank per tok-chunk
                    for t in range(4):
                        vps = v_ps.tile([128, 256], f32, tag="vps")
                        for kc in range(NK):
                            nc.tensor.matmul(
                                vps,
                                x_sb[:, kc, 128 * t : 128 * (t + 1)],
                                w_sb[:, kc, 512:768],
                                start=(kc == 0),
                                stop=(kc == NK - 1),
                            )
                        nc.scalar.activation(
                            out=v_res[:, 4 * a + t, :],
                            in_=vps,
                            func=mybir.ActivationFunctionType.Copy,
                        )

            if debug:
                nc.sync.dma_start(out=dbg_qk[:, :], in_=qk_res[:, :, :])
                nc.sync.dma_start(out=dbg_v[:, :], in_=v_res[:, :, :])

            # ------- phases 2+3: attention + out-projection per batch -------
            with (
                tc.tile_pool(name="ptp", bufs=20) as ptp,
                tc.tile_pool(name="zaccp", bufs=3) as zaccp,
                tc.tile_pool(name="zrp", bufs=3) as zrp,
                tc.tile_pool(name="wvnp", bufs=2) as wvnp,
                tc.tile_pool(name="ostage", bufs=4) as ostage,
                tc.tile_pool(name="sc_ps", bufs=3, space="PSUM") as sc_ps,
                tc.tile_pool(name="z_ps", bufs=1, space="PSUM") as z_ps,
                tc.tile_pool(name="wv_ps", bufs=2, space="PSUM") as wv_ps,
                tc.tile_pool(name="o_ps", bufs=2, space="PSUM") as o_ps,
            ):
                for b in range(B):
                    wvn_tiles = [
                        wvnp.tile([128, S], f16, tag=f"wvn{h}", name=f"wvn{h}")
                        for h in range(HPC)
                    ]
                    for ast in range(S // 512):
                        for h in range(HPC):
                            q_sb = qk_res[:, h, S * b + 512 * ast : S * b + 512 * (ast + 1)]
                            koff = 2 + h
                            wv = wv_ps.tile([128, 512], f32, tag="wv")
                            z_acc = zaccp.tile([128, 512], f16, tag="zacc")
                            nfull = 4 * ast
                            pts = []
                            # full t-blocks
                            for j in range(nfull):
                                scp = sc_ps.tile([128, 512], f32, tag="sc")
                                nc.tensor.matmul(
                                    scp,
                                    qk_res[
                                        :, koff,
                                        S * b + 128 * j : S * b + 128 * (j + 1),
                                    ],
                                    q_sb,
                                    start=True,
                                    stop=True,
                                )
                                pt = ptp.tile([128, 512], f16, tag="pt")
                                nc.scalar.activation(
                                    out=pt,
                                    in_=scp,
                                    func=mybir.ActivationFunctionType.Exp,
                                    scale=SCALE,
                                )
                                pts.append(pt)
                                if j == 0:
                                    nc.vector.tensor_copy(out=z_acc, in_=pt)
                                else:
                                    nc.vector.tensor_add(z_acc, z_acc, pt)
                            # diagonal t-blocks: sliced to the causal region
                            for d in range(4):
                                j = nfull + d
                                lo = 128 * d
                                scd = sc_ps.tile([128, 512], f32, tag="sc")
                                nc.tensor.matmul(
                                    scd[:, lo:512],
                                    qk_res[
                                        :, koff,
                                        S * b + 128 * j : S * b + 128 * (j + 1),
                                    ],
                                    q_sb[:, lo:512],
                                    start=True,
                                    stop=True,
                                )
                                ptd = ptp.tile([128, 512], f16, tag="pt")
                                nc.scalar.activation(
                                    out=ptd[:, lo:512],
                                    in_=scd[:, lo:512],
                                    func=mybir.ActivationFunctionType.Exp,
                                    scale=SCALE,
                                )
                                nc.vector.tensor_mul(
                                    ptd[:, lo : lo + 128],
                                    ptd[:, lo : lo + 128],
                                    tri,
                                )
                                pts.append(ptd)
                                if nfull == 0 and d == 0:
                                    nc.vector.tensor_copy(
                                        out=z_acc, in_=ptd[:, 0:512]
                                    )
                                else:
                                    nc.vector.tensor_add(
                                        z_acc[:, lo:512],
                                        z_acc[:, lo:512],
                                        ptd[:, lo:512],
                                    )
                            # wv accumulation (after all sc/exp emitted so PE
                            # has lookahead while ScalarE works on the exps)
                            for j in range(nfull):
                                nc.tensor.matmul(
                                    wv,
                                    v_res[
                                        :, 16 * b + j, 128 * h : 128 * (h + 1)
                                    ],
                                    pts[j],
                                    start=(j == 0),
                                    stop=False,
                                )
                            for d in range(4):
                                j = nfull + d
                                lo = 128 * d
                                ptd = pts[j]
                                nc.tensor.matmul(
                                    wv[:, lo:512],
                                    v_res[
                                        :, 16 * b + j, 128 * h : 128 * (h + 1)
                                    ],
                                    ptd[:, lo:512],
                                    start=(nfull == 0 and d == 0),
                                    stop=(d == 3),
                                    skip_group_check=True,
                                )
                            # normalize: z broadcast via ones-matmul, recip, mul
                            zs = z_ps.tile([128, 512], f32, tag="zs")
                            nc.tensor.matmul(
                                zs, ones_mat, z_acc, start=True, stop=True
                            )
                            zr = zrp.tile([128, 512], f16, tag="zr")
                            with nc.allow_low_precision(
                                reason="softmax denom reciprocal in fp16"
                            ):
                                nc.vector.reciprocal(out=zr, in_=zs)
                                nc.vector.tensor_mul(
                                    wvn_tiles[h][:, 512 * ast : 512 * (ast + 1)],
                                    wv,
                                    zr,
                                )
                        if debug and ast == S // 512 - 1:
                            for h in range(HPC):
                                nc.sync.dma_start(
                                    out=dbg_wvn[
                                        :,
                                        (b * HPC + h) * S : (b * HPC + h + 1) * S,
                                    ],
                                    in_=wvn_tiles[h][:, :],
                                )
                        # out-projection for this 512-token group
                        for tk in range(4 * ast, 4 * (ast + 1)):
                            ost = ostage.tile([128, NS], f16, tag="ost")
                            for nh in range(4):
                                ops = o_ps.tile([128, 512], f32, tag="ops")
                                for h in range(HPC):
                                    nc.tensor.matmul(
                                        ops,
                                        wvn_tiles[h][:, 128 * tk : 128 * (tk + 1)],
                                        wout_sb[:, h, 512 * nh : 512 * (nh + 1)],
                                        start=(h == 0),
                                        stop=(h == HPC - 1),
                                    )
                                if nh % 2 == 0:
                                    nc.vector.tensor_copy(
                                        out=ost[:, 512 * nh : 512 * (nh + 1)],
                                        in_=ops,
                                    )
                                else:
                                    nc.scalar.activation(
                                        out=ost[:, 512 * nh : 512 * (nh + 1)],
                                        in_=ops,
                                        func=mybir.ActivationFunctionType.Copy,
                                    )
                            nc.sync.dma_start(
                                out=outp[
                                    S * b + 128 * tk : S * b + 128 * (tk + 1), :
                                ],
                                in_=ost,
                            )

    nc.compile()
    return nc


def _causal_fastpath_ok(mask, cache_pos):
    if cache_pos.shape != (S,) or not np.array_equal(
        np.asarray(cache_pos), np.arange(S, dtype=np.int64).astype(cache_pos.dtype)
    ):
        return False
    m = np.asarray(mask).reshape(S, T)
    rows = np.arange(S)[:, None]
    cols = np.arange(T)[None, :]
    return np.array_equal(m, cols <= rows)


def _numpy_fallback(input_ids, mask, cache_pos, w_qkv, w_out, k_cache, v_cache):
    x = np.asarray(input_ids, dtype=np.float32)
    qkv = np.einsum("bsd,ed->bse", x, np.asarray(w_qkv, np.float32))
    q, k, v = np.split(qkv, 3, axis=-1)

    def heads(t):
        return t.reshape(B, S, H, DH).transpose(0, 2, 1, 3)

    q, k, v = heads(q), heads(k), heads(v)
    kf = np.array(k_cache, np.float32)
    vf = np.array(v_cache, np.float32)
    kf[:, :, np.asarray(cache_pos)] = k
    vf[:, :, np.asarray(cache_pos)] = v
    sc = np.einsum("bhsd,bhtd->bhst", q, kf) * SCALE
    sc = np.where(np.asarray(mask), sc, np.finfo(np.float32).min)
    sc = sc - sc.max(axis=-1, keepdims=True)
    p = np.exp(sc)
    p = p / p.sum(axis=-1, keepdims=True)
    wv = np.einsum("bhst,bhtd->bhsd", p, vf)
    wv = wv.transpose(0, 2, 1, 3).reshape(B, S, NS)
    return np.einsum("bsd,ed->bse", wv, np.asarray(w_out, np.float32))


def _build_cmask_host():
    # [tri | ones]: tri[t, s'] = 1.0 where s' >= t (within a 128x128 block)
    t = np.arange(128)[:, None]
    s = np.arange(128)[None, :]
    tri = (s >= t).astype(np.float16)
    ones = np.ones((128, 128), dtype=np.float16)
    return np.concatenate([tri, ones], axis=1)  # [128, 256]


def _run_on_device(in_maps, trace=False):
    from concourse.bass_utils import run_bass_kernel_spmd

    if "nc" not in _CACHED:
        _CACHED["nc"] = _build_program()
    nc = _CACHED["nc"]
    return run_bass_kernel_spmd(
        nc, in_maps, core_ids=list(range(NCORES)), trace=trace
    )


def _prep_in_maps(input_ids, w_qkv, w_out):
    x2d = np.asarray(input_ids, np.float32).reshape(TOK, NS).T.astype(np.float16)
    # permute rows so a [128, 4, 512] DMA from rows [512g, 512g+512) lands as
    # x_sb[p, 4g+k, :] = x2d[512g + 128k + p, :]  (DMA fills partition-major:
    # dram row 512g + 4p + k -> x_sb[p, 4g+k]).
    x2d = np.ascontiguousarray(
        x2d.reshape(4, 4, 128, TOK).transpose(0, 2, 1, 3).reshape(NS, TOK)
    )  # [NS, TOK] fp16, row-permuted
    cm = _build_cmask_host()
    wq = np.asarray(w_qkv, np.float32)
    wo = np.asarray(w_out, np.float32)
    in_maps = []
    for c in range(NCORES):
        lo, hi = c * DPC, (c + 1) * DPC
        w_slice = np.concatenate(
            [wq[lo:hi], wq[NS + lo : NS + hi], wq[2 * NS + lo : 2 * NS + hi]],
            axis=0,
        )  # [768, NS] (q,k,v rows for this core's heads)
        wT_c = np.ascontiguousarray(w_slice.T.astype(np.float16))     # [NS, 768]
        woutT_c = np.ascontiguousarray(wo[:, lo:hi].T.astype(np.float16))
        in_maps.append({"xT": x2d, "wT": wT_c, "woutT": woutT_c, "cmask": cm})
    return in_maps


def kernel(input_ids, mask, cache_pos, w_qkv, w_out, k_cache, v_cache):
    if not _causal_fastpath_ok(mask, cache_pos):
        return _numpy_fallback(
            input_ids, mask, cache_pos, w_qkv, w_out, k_cache, v_cache
        )
    in_maps = _prep_in_maps(input_ids, w_qkv, w_out)
    res = _run_on_device(in_maps)
    out = np.zeros((TOK, NS), np.float32)
    for r in res.results:
        out += r["outp"].astype(np.float32)
    return out.reshape(B, S, NS)


# revision 31
# speedup vs baseline: 1.2341x; 1.0124x over previous
"""Trainium2 Bass kernel for nn_CausalSelfAttention_40810779247124.

Head-sharded (tensor-parallel) causal self-attention prefill across 8
NeuronCores: 2 heads per core, both batches on every core.  All matmul
operands are fp16 (fp32 PSUM accumulation), which runs at the full
1 row/cycle PE rate while halving DMA traffic and enabling the DVE
2x mode for elementwise work.

Per core:
  phase 1: QKV projection.  Q,K kept resident in SBUF in [e, tok]
           layout; V resident in [tok, e] layout (both fp16).
  phase 2: attention, transposed: scoresT[t,s] = K.T @ Q on PE,
           exp on ScalarE (PSUM f32 -> SBUF fp16), softmax denominator
           accumulated on DVE (fp16 2x adds) + one ones-matmul
           partition-reduce/broadcast per s-tile, wvT[e,s] = V.T @ P.T
           accumulated in PSUM.  Causal structure exploited at 128-token
           granularity via sliced matmuls on the diagonal s-tiles.
  phase 3: output projection partial for this core's 256-dim slice,
           interleaved with attention per 512-token group; the
           all-reduce over cores happens on the host (sum of 8 fp16
           partials in f32) during unsharding.

The host verifies that mask/cache_pos match the causal-prefill pattern
and falls back to a numpy reference otherwise.
"""

import sys

sys.path.insert(0, "/opt/trn_rl_repo")

import numpy as np

B = 2
S = 2048
T = 4096
NS = 2048          # n_state
H = 16
DH = 128
NCORES = 8
HPC = H // NCORES  # heads per core = 2
DPC = HPC * DH     # d-slice per core = 256
TOK = B * S        # 4096 tokens across batches
SCALE = 1.0 / float(np.sqrt(DH))

_CACHED = {}


def _build_program(debug=False):
    import concourse.bacc as bacc
    import concourse.bass as bass
    import concourse.tile as tile
    from concourse import mybir
    f16 = mybir.dt.float16
    f32 = mybir.dt.float32

    nc = bacc.Bacc()

    xT = nc.dram_tensor("xT", [NS, TOK], f16, kind="ExternalInput")
    wT = nc.dram_tensor("wT", [NS, 6 * DH], f16, kind="ExternalInput")
    woutT = nc.dram_tensor("woutT", [DPC, NS], f16, kind="ExternalInput")
    cmask = nc.dram_tensor("cmask", [DH, 256], f16, kind="ExternalInput")
    outp = nc.dram_tensor("outp", [TOK, NS], f16, kind="ExternalOutput")
    if debug:
        dbg_qk = nc.dram_tensor("dbg_qk", [128, 4 * TOK], f16, kind="ExternalOutput")
        dbg_v = nc.dram_tensor("dbg_v", [128, (TOK // 128) * DPC], f16, kind="ExternalOutput")
        dbg_wvn = nc.dram_tensor("dbg_wvn", [128, B * HPC * S], f16, kind="ExternalOutput")

    NT = TOK // 512   # 8 tok-tiles of 512
    NK = NS // 128    # 16 contraction chunks

    with tile.TileContext(nc) as tc:
        with (
            tc.tile_pool(name="constp", bufs=1) as constp,
            tc.tile_pool(name="vresp", bufs=1) as vresp,
            tc.tile_pool(name="qkresp", bufs=1) as qkresp,
            tc.tile_pool(name="wp", bufs=1) as wp,
            tc.tile_pool(name="woutp", bufs=1) as woutp,
        ):
            cm_sb = constp.tile([DH, 256], f16)
            tri = cm_sb[:, 0:128]     # tri[t, s'] = s' >= t
            ones_mat = cm_sb[:, 128:256]

            # residents: V[tok, e] (32 chunks of 128 tok), Q/K [e, tok]
            v_res = vresp.tile([128, TOK // 128, DPC], f16)
            qk_res = qkresp.tile([128, 4, TOK], f16)  # m = q0,q1,k0,k1
            w_sb = wp.tile([128, NK, 6 * DH], f16)
            wout_sb = woutp.tile([128, HPC, NS], f16)

            # ---------------- phase 1: QKV projection ----------------
            with (
                tc.tile_pool(name="xp", bufs=2) as xp,
                tc.tile_pool(name="qkv_ps", bufs=4, space="PSUM") as qkv_ps,
                tc.tile_pool(name="v_ps", bufs=4, space="PSUM") as v_ps,
            ):
                x_next = None
                for a in range(NT):
                    if a == 1:
                        # data prefetched into x_next during a=0's DMA stream
                        x_sb = x_next
                    else:
                        x_sb = xp.tile([128, NK, 512], f16, tag="x_sb")
                    # x DMAs are [128, 4, 512]; the host pre-permutes xT rows
                    # (see _prep_in_maps) so group g row 4*p+k holds logical
                    # row 128*k+p and the tile lands as x_sb[p, kc, c] =
                    # x[128*kc+p, col].
                    if a == 0:
                        # DMA issue order tuned so each consumer's data lands
                        # just ahead of use: w kc=0 + x kc=0 first (compute
                        # starts after ~1us), then w chunks interleaved with
                        # x group prefetches for a=0 tail and a=1.
                        nc.sync.dma_start(
                            out=w_sb[:, 0, :], in_=wT[0:128, :]
                        )
                        # kc=0..3 chunks via strided rows (4p+k), finest first
                        for k in range(4):
                            nc.sync.dma_start(
                                out=x_sb[:, k, :],
                                in_=xT[k:512:4, 0:512],
                            )
                        x_next = xp.tile([128, NK, 512], f16, tag="x_sb",
                                         name="x_next")
                        for q in range(1, 4):
                            for kc in range(3 * q - 2, 3 * q + 1):
                                nc.sync.dma_start(
                                    out=w_sb[:, kc, :],
                                    in_=wT[128 * kc : 128 * (kc + 1), :],
                                )
                            nc.sync.dma_start(
                                out=x_sb[:, 4 * q : 4 * (q + 1), :],
                                in_=xT[512 * q : 512 * (q + 1), 0:512],
                            )
                        for kc in range(10, 16):
                            nc.sync.dma_start(
                                out=w_sb[:, kc, :],
                                in_=wT[128 * kc : 128 * (kc + 1), :],
                            )
                        for q in range(4):
                            nc.sync.dma_start(
                                out=x_next[:, 4 * q : 4 * (q + 1), :],
                                in_=xT[512 * q : 512 * (q + 1), 512:1024],
                            )
                        nc.sync.dma_start(out=cm_sb, in_=cmask[:, :])
                        for h in range(HPC):
                            nc.sync.dma_start(
                                out=wout_sb[:, h, :],
                                in_=woutT[128 * h : 128 * (h + 1), :],
                            )
                        # kc-outer so each chunk's matmuls only need that
                        # chunk's x/w DMA
                        pss4 = [
                            qkv_ps.tile([128, 512], f32, tag="qkv", name=f"pss{m}")
                            for m in range(4)
                        ]
                        vps4 = [
                            v_ps.tile([128, 256], f32, tag="vps", name=f"vps{t}")
                            for t in range(4)
                        ]
                        for kc in range(NK):
                            for m in range(4):
                                nc.tensor.matmul(
                                    pss4[m],
                                    w_sb[:, kc, 128 * m : 128 * (m + 1)],
                                    x_sb[:, kc, :],
                                    start=(kc == 0),
                                    stop=(kc == NK - 1),
                                )
                            for t in range(4):
                                nc.tensor.matmul(
                                    vps4[t],
                                    x_sb[:, kc, 128 * t : 128 * (t + 1)],
                                    w_sb[:, kc, 512:768],
                                    start=(kc == 0),
                                    stop=(kc == NK - 1),
                                )
                        for m in range(4):
                            nc.scalar.activation(
                                out=qk_res[:, m, 0:512],
                                in_=pss4[m],
                                func=mybir.ActivationFunctionType.Copy,
                            )
                        for t in range(4):
                            nc.scalar.activation(
                                out=v_res[:, t, :],
                                in_=vps4[t],
                                func=mybir.ActivationFunctionType.Copy,
                            )
                        continue
                    if a != 1:
                        for q in range(4):
                            nc.sync.dma_start(
                                out=x_sb[:, 4 * q : 4 * (q + 1), :],
                                in_=xT[
                                    512 * q : 512 * (q + 1),
                                    512 * a : 512 * (a + 1),
                                ],
                            )
                    # Q,K: out [e(128), tok(512)] per m-tile
                    for m in range(4):
                        pss = qkv_ps.tile([128, 512], f32, tag="qkv")
                        for kc in range(NK):
                            nc.tensor.matmul(
                                pss,
                                w_sb[:, kc, 128 * m : 128 * (m + 1)],
                                x_sb[:, kc, :],
                                start=(kc == 0),
                                stop=(kc == NK - 1),
                            )
                        nc.scalar.activation(
                            out=qk_res[:, m, 512 * a : 512 * (a + 1)],
                            in_=pss,
                            func=mybir.ActivationFunctionType.Copy,
                        )
                    # V: out [tok(128), e(256)], one bank per tok-chunk
                    for t in range(4):
                        vps = v_ps.tile([128, 256], f32, tag="vps")
                        for kc in range(NK):
                            nc.tensor.matmul(
                                vps,
                                x_sb[:, kc, 128 * t : 128 * (t + 1)],
                                w_sb[:, kc, 512:768],
                                start=(kc == 0),
                                stop=(kc == NK - 1),
                            )
                        nc.scalar.activation(
                            out=v_res[:, 4 * a + t, :],
                            in_=vps,
                            func=mybir.ActivationFunctionType.Copy,
                        )

            if debug:
                nc.sync.dma_start(out=dbg_qk[:, :], in_=qk_res[:, :, :])
                nc.sync.dma_start(out=dbg_v[:, :], in_=v_res[:, :, :])

            # ------- phases 2+3: attention + out-projection per batch -------
            with (
                tc.tile_pool(name="ptp", bufs=36) as ptp,
                tc.tile_pool(name="zaccp", bufs=3) as zaccp,
                tc.tile_pool(name="zrp", bufs=3) as zrp,
                tc.tile_pool(name="wvnp", bufs=2) as wvnp,
                tc.tile_pool(name="ostage", bufs=4) as ostage,
                tc.tile_pool(name="sc_ps", bufs=3, space="PSUM") as sc_ps,
                tc.tile_pool(name="z_ps", bufs=1, space="PSUM") as z_ps,
                tc.tile_pool(name="wv_ps", bufs=2, space="PSUM") as wv_ps,
                tc.tile_pool(name="o_ps", bufs=2, space="PSUM") as o_ps,
            ):
                for b in range(B):
                    wvn_tiles = [
                        wvnp.tile([128, S], f16, tag=f"wvn{h}", name=f"wvn{h}")
                        for h in range(HPC)
                    ]
                    for ast in range(S // 512):
                        nfull = 4 * ast
                        hstate = []
                        # both heads' score/exp/z streams first: ScalarE gets a
                        # deep exp backlog while PE streams matmuls
                        for h in range(HPC):
                            q_sb = qk_res[:, h, S * b + 512 * ast : S * b + 512 * (ast + 1)]
                            koff = 2 + h
                            z_acc = zaccp.tile([128, 512], f16, tag="zacc")
                            pts = []
                            for j in range(nfull):
                                scp = sc_ps.tile([128, 512], f32, tag="sc")
                                nc.tensor.matmul(
                                    scp,
                                    qk_res[
                                        :, koff,
                                        S * b + 128 * j : S * b + 128 * (j + 1),
                                    ],
                                    q_sb,
                                    start=True,
                                    stop=True,
                                )
                                pt = ptp.tile([128, 512], f16, tag="pt")
                                nc.scalar.activation(
                                    out=pt,
                                    in_=scp,
                                    func=mybir.ActivationFunctionType.Exp,
                                    scale=SCALE,
                                )
                                pts.append(pt)
                                if j == 0:
                                    nc.vector.tensor_copy(out=z_acc, in_=pt)
                                else:
                                    nc.vector.tensor_add(z_acc, z_acc, pt)
                            # diagonal t-blocks: sliced to the causal region
                            for d in range(4):
                                j = nfull + d
                                lo = 128 * d
                                scd = sc_ps.tile([128, 512], f32, tag="sc")
                                nc.tensor.matmul(
                                    scd[:, lo:512],
                                    qk_res[
                                        :, koff,
                                        S * b + 128 * j : S * b + 128 * (j + 1),
                                    ],
                                    q_sb[:, lo:512],
                                    start=True,
                                    stop=True,
                                )
                                ptd = ptp.tile([128, 512], f16, tag="pt")
                                nc.scalar.activation(
                                    out=ptd[:, lo:512],
                                    in_=scd[:, lo:512],
                                    func=mybir.ActivationFunctionType.Exp,
                                    scale=SCALE,
                                )
                                nc.vector.tensor_mul(
                                    ptd[:, lo : lo + 128],
                                    ptd[:, lo : lo + 128],
                                    tri,
                                )
                                pts.append(ptd)
                                if nfull == 0 and d == 0:
                                    nc.vector.tensor_copy(
                                        out=z_acc, in_=ptd[:, 0:512]
                                    )
                                else:
                                    nc.vector.tensor_add(
                                        z_acc[:, lo:512],
                                        z_acc[:, lo:512],
                                        ptd[:, lo:512],
                                    )
                            hstate.append((pts, z_acc))
                        # wv accumulation streams
                        for h in range(HPC):
                            pts, z_acc = hstate[h]
                            wv = wv_ps.tile([128, 512], f32, tag="wv")
                            hstate[h] = (pts, z_acc, wv)
                            for j in range(nfull):
                                nc.tensor.matmul(
                                    wv,
                                    v_res[
                                        :, 16 * b + j, 128 * h : 128 * (h + 1)
                                    ],
                                    pts[j],
                                    start=(j == 0),
                                    stop=False,
                                )
                            for d in range(4):
                                j = nfull + d
                                lo = 128 * d
                                ptd = pts[j]
                                nc.tensor.matmul(
                                    wv[:, lo:512],
                                    v_res[
                                        :, 16 * b + j, 128 * h : 128 * (h + 1)
                                    ],
                                    ptd[:, lo:512],
                                    start=(nfull == 0 and d == 0),
                                    stop=(d == 3),
                                    skip_group_check=True,
                                )
                        # normalize: z broadcast via ones-matmul, recip, mul
                        for h in range(HPC):
                            pts, z_acc, wv = hstate[h]
                            zs = z_ps.tile([128, 512], f32, tag="zs")
                            nc.tensor.matmul(
                                zs, ones_mat, z_acc, start=True, stop=True
                            )
                            zr = zrp.tile([128, 512], f16, tag="zr")
                            with nc.allow_low_precision(
                                reason="softmax denom reciprocal in fp16"
                            ):
                                nc.vector.reciprocal(out=zr, in_=zs)
                                nc.vector.tensor_mul(
                                    wvn_tiles[h][:, 512 * ast : 512 * (ast + 1)],
                                    wv,
                                    zr,
                                )
                        if debug and ast == S // 512 - 1:
                            for h in range(HPC):
                                nc.sync.dma_start(
                                    out=dbg_wvn[
                                        :,
                                        (b * HPC + h) * S : (b * HPC + h + 1) * S,
                                    ],
                                    in_=wvn_tiles[h][:, :],
                                )
                        # out-projection for this 512-token group
                        final = b == B - 1 and ast == S // 512 - 1
                        for tk in range(4 * ast, 4 * (ast + 1)):
                            ost = ostage.tile([128, NS], f16, tag="ost")
                            for nh in range(4):
                                ops = o_ps.tile([128, 512], f32, tag="ops")
                                for h in range(HPC):
                                    nc.tensor.matmul(
                                        ops,
                                        wvn_tiles[h][:, 128 * tk : 128 * (tk + 1)],
                                        wout_sb[:, h, 512 * nh : 512 * (nh + 1)],
                                        start=(h == 0),
                                        stop=(h == HPC - 1),
                                    )
                                if nh % 2 == 0:
                                    nc.vector.tensor_copy(
                                        out=ost[:, 512 * nh : 512 * (nh + 1)],
                                        in_=ops,
                                    )
                                else:
                                    nc.scalar.activation(
                                        out=ost[:, 512 * nh : 512 * (nh + 1)],
                                        in_=ops,
                                        func=mybir.ActivationFunctionType.Copy,
                                    )
                            if final and tk == 4 * ast + 3:
                                nc.sync.dma_start(
                                    out=outp[
                                        S * b + 128 * tk : S * b + 128 * (tk + 1),
                                        0:1024,
                                    ],
                                    in_=ost[:, 0:1024],
                                )
                                nc.sync.dma_start(
                                    out=outp[
                                        S * b + 128 * tk : S * b + 128 * (tk + 1),
                                        1024:2048,
                                    ],
                                    in_=ost[:, 1024:2048],
                                )
                            else:
                                nc.sync.dma_start(
                                    out=outp[
                                        S * b + 128 * tk : S * b + 128 * (tk + 1), :
                                    ],
                                    in_=ost,
                                )

    nc.compile()
    return nc


def _causal_fastpath_ok(mask, cache_pos):
    if cache_pos.shape != (S,) or not np.array_equal(
        np.asarray(cache_pos), np.arange(S, dtype=np.int64).astype(cache_pos.dtype)
    ):
        return False
    m = np.asarray(mask).reshape(S, T)
    rows = np.arange(S)[:, None]
    cols = np.arange(T)[None, :]
    return np.array_equal(m, cols <= rows)


def _numpy_fallback(input_ids, mask, cache_pos, w_qkv, w_out, k_cache, v_cache):
    x = np.asarray(input_ids, dtype=np.float32)
    qkv = np.einsum("bsd,ed->bse", x, np.asarray(w_qkv, np.float32))
    q, k, v = np.split(qkv, 3, axis=-1)

    def heads(t):
        return t.reshape(B, S, H, DH).transpose(0, 2, 1, 3)

    q, k, v = heads(q), heads(k), heads(v)
    kf = np.array(k_cache, np.float32)
    vf = np.array(v_cache, np.float32)
    kf[:, :, np.asarray(cache_pos)] = k
    vf[:, :, np.asarray(cache_pos)] = v
    sc = np.einsum("bhsd,bhtd->bhst", q, kf) * SCALE
    sc = np.where(np.asarray(mask), sc, np.finfo(np.float32).min)
    sc = sc - sc.max(axis=-1, keepdims=True)
    p = np.exp(sc)
    p = p / p.sum(axis=-1, keepdims=True)
    wv = np.einsum("bhst,bhtd->bhsd", p, vf)
    wv = wv.transpose(0, 2, 1, 3).reshape(B, S, NS)
    return np.einsum("bsd,ed->bse", wv, np.asarray(w_out, np.float32))


def _build_cmask_host():
    # [tri | ones]: tri[t, s'] = 1.0 where s' >= t (within a 128x128 block)
    t = np.arange(128)[:, None]
    s = np.arange(128)[None, :]
    tri = (s >= t).astype(np.float16)
    ones = np.ones((128, 128), dtype=np.float16)
    return np.concatenate([tri, ones], axis=1)  # [128, 256]


def _run_on_device(in_maps, trace=False):
    from concourse.bass_utils import run_bass_kernel_spmd

    if "nc" not in _CACHED:
        _CACHED["nc"] = _build_program()
    nc = _CACHED["nc"]
    return run_bass_kernel_spmd(
        nc, in_maps, core_ids=list(range(NCORES)), trace=trace
    )


def _prep_in_maps(input_ids, w_qkv, w_out):
    x2d = np.asarray(input_ids, np.float32).reshape(TOK, NS).T.astype(np.float16)
    # permute rows so a [128, 4, 512] DMA from rows [512g, 512g+512) lands as
    # x_sb[p, 4g+k, :] = x2d[512g + 128k + p, :]  (DMA fills partition-major:
    # dram row 512g + 4p + k -> x_sb[p, 4g+k]).
    x2d = np.ascontiguousarray(
        x2d.reshape(4, 4, 128, TOK).transpose(0, 2, 1, 3).reshape(NS, TOK)
    )  # [NS, TOK] fp16, row-permuted
    cm = _build_cmask_host()
    wq = np.asarray(w_qkv, np.float32)
    wo = np.asarray(w_out, np.float32)
    in_maps = []
    for c in range(NCORES):
        lo, hi = c * DPC, (c + 1) * DPC
        w_slice = np.concatenate(
            [wq[lo:hi], wq[NS + lo : NS + hi], wq[2 * NS + lo : 2 * NS + hi]],
            axis=0,
        )  # [768, NS] (q,k,v rows for this core's heads)
        wT_c = np.ascontiguousarray(w_slice.T.astype(np.float16))     # [NS, 768]
        woutT_c = np.ascontiguousarray(wo[:, lo:hi].T.astype(np.float16))
        in_maps.append({"xT": x2d, "wT": wT_c, "woutT": woutT_c, "cmask": cm})
    return in_maps


def kernel(input_ids, mask, cache_pos, w_qkv, w_out, k_cache, v_cache):
    if not _causal_fastpath_ok(mask, cache_pos):
        return _numpy_fallback(
            input_ids, mask, cache_pos, w_qkv, w_out, k_cache, v_cache
        )
    in_maps = _prep_in_maps(input_ids, w_qkv, w_out)
    res = _run_on_device(in_maps)
    out = np.zeros((TOK, NS), np.float32)
    for r in res.results:
        out += r["outp"].astype(np.float32)
    return out.reshape(B, S, NS)
